# revision 22
# baseline (speedup 1.0000x reference)
"""Distributed GQA attention (B=2,S=2048,H=2048,NH=16,NKV=4,HD=128) on 8 TRN2 cores.

Strategy: tensor-parallel over heads (2 Q heads + 1 KV head per core).
Each core streams x once in a per-core order (its kv half first): the
first 4 pos-tiles produce K^T and V^T for its half (512-wide matmuls; V is
fixed up by a DMA transpose) plus Q; the rest produce Q only. K/V halves
are exchanged with a pairwise AllGather (even rank = batch 0, so the
gathered layout is globally batch-ordered); Q is rearranged to global
batch order with mask selects driven by a per-core 0/1 input. Causal
flash attention runs in scores-transposed layout with kt-paired exp
instructions and pair-summed softmax denominators. An AllToAll per
head-half switches to sequence-parallel o_proj.
"""

import contextlib
import math

import numpy as np
import ml_dtypes

import concourse.bass as bass
import concourse.mybir as mybir
import concourse.tile as tile
from concourse.tile import add_dep_helper
from concourse import bacc
from concourse.bass_utils import run_bass_kernel_spmd
from concourse.masks import make_identity

BF16 = mybir.dt.bfloat16
F32 = mybir.dt.float32

B, S, H = 2, 2048, 2048
NH, NKV, HD = 16, 4, 128
NCORES = 8
HPC = NH // NCORES          # q heads per core = 2
POS = B * S                 # 4096 flattened rows
RPC = POS // NCORES         # output rows per core = 512
KT = H // 128               # 16 contraction tiles for projections
PT_N = POS // 512           # 8 pos-tiles of 512
HPT = PT_N // 2             # pos-tiles in my kv half = 4
SCALE = 1.0 / math.sqrt(HD)

_CACHE = {}


def _build():
    nc = bacc.Bacc("TRN2", target_bir_lowering=False, debug=False,
                   num_devices=NCORES)

    # all parameters are host-prepacked partition-major so every DMA is a
    # contiguous-per-partition 2D block (cheap DIRECT2D issue, few
    # descriptors)
    xT = nc.declare_dram_parameter("xT", [PT_N, 128, KT, 512], BF16,
                                   isOutput=False)
    wq = nc.declare_dram_parameter("wq", [128, KT, HPC * HD], BF16,
                                   isOutput=False)
    wk = nc.declare_dram_parameter("wk", [128, KT, HD], BF16, isOutput=False)
    wv = nc.declare_dram_parameter("wv", [128, KT, HD], BF16, isOutput=False)
    cosT = nc.declare_dram_parameter("cosT", [HD, S], BF16, isOutput=False)
    ssinT = nc.declare_dram_parameter("ssinT", [HD, S], BF16, isOutput=False)
    qsel = nc.declare_dram_parameter("qsel", [128, 512], mybir.dt.uint8,
                                     isOutput=False)
    wo = nc.declare_dram_parameter("wo", [2, 128, KT // 2, H], BF16,
                                   isOutput=False)
    out = nc.declare_dram_parameter("out", [RPC, H], BF16,
                                    isOutput=True)

    unit_last = [None]
    unit_first = [None]
    unit_latest = [None]

    def pe(mm):
        # chain PE work at unit granularity: the first matmul of each unit
        # depends on the last matmul of the previous unit; within a unit the
        # scheduler is free to pipeline.
        if unit_first[0] is None:
            unit_first[0] = mm
            if unit_last[0] is not None:
                add_dep_helper(mm.ins, unit_last[0].ins, False)
        unit_latest[0] = mm
        return mm

    def close_unit():
        unit_last[0] = unit_latest[0]
        unit_first[0] = None

    vunit_last = [None]
    vunit_first = [None]
    vunit_latest = [None]

    def ve(op):
        # same unit-granularity chain for the vector queue, so late critical
        # ops (stt/recip) aren't scheduled behind later units' vector work
        if vunit_first[0] is None:
            vunit_first[0] = op
            if vunit_last[0] is not None:
                add_dep_helper(op.ins, vunit_last[0].ins, False)
        vunit_latest[0] = op
        return op

    def close_vunit():
        vunit_last[0] = vunit_latest[0]
        vunit_first[0] = None

    with tile.TileContext(nc) as tc:
        with (
            tc.tile_pool(name="const", bufs=1) as const,
            tc.tile_pool(name="wpool", bufs=1) as wpool,
            tc.tile_pool(name="qkv", bufs=1) as qkv,
            tc.tile_pool(name="dram", bufs=1, space="DRAM") as dram,
        ):
            # warmup scratch first: one cheap vector memset unblocks the
            # HAM-warmup matmuls immediately
            wrm = const.tile([128, 128], BF16)
            nc.vector.memset(wrm, 0.5)

            # kv-proj weights first on gpsimd: needed by the very first
            # matmuls
            wk_sb = wpool.tile([128, KT, HD], BF16)
            wv_sb = wpool.tile([128, KT, HD], BF16)
            nc.gpsimd.dma_start(wk_sb[:], wk.ap())
            nc.gpsimd.dma_start(wv_sb[:], wv.ap())

            # ---- constants / weights resident in SBUF ----
            ident = const.tile([128, 128], BF16)
            make_identity(nc, ident)
            # upper-triangular (incl diag) mask: valid where kpos <= q
            triT = const.tile([128, 128], BF16)
            nc.gpsimd.memset(triT, 1.0)
            nc.gpsimd.affine_select(
                out=triT, in_=triT, compare_op=mybir.AluOpType.is_ge,
                fill=0.0, base=0, pattern=[[1, 128]], channel_multiplier=-1,
            )  # keep 1.0 where (c - p) >= 0, i.e. kpos <= q
            ones_sb = const.tile([128, 128], BF16)
            nc.gpsimd.memset(ones_sb, 1.0)

            cos_sb = const.tile([128, S], BF16)
            sin_sb = const.tile([128, S], BF16)
            qsel_sb = const.tile([128, 512], mybir.dt.uint8)

            wq_sb = wpool.tile([128, KT, HPC * HD], BF16)
            woe_sb = wpool.tile([128, KT // 2, H], BF16)
            woo_sb = wpool.tile([128, KT // 2, H], BF16)

            # persistent q/k/v for both batches, global batch order (bf16)
            q_all = qkv.tile([128, HPC, POS], BF16)
            kT_all = qkv.tile([128, POS], BF16)
            v_all = qkv.tile([128, POS // 128, HD], BF16)

            # single merged kv exchange:
            # [0]=kT half a, [1]=kT half b, [2]=v half a, [3]=v half b
            exch_in = dram.tile([4, 128, 1024], BF16)
            exch_out = dram.tile([2, 4, 128, 1024], BF16)
            a2a_in1 = dram.tile([NCORES, HD, RPC], BF16)
            a2a_out1 = dram.tile([NCORES, HD, RPC], BF16)
            a2a_in2 = dram.tile([NCORES, HD, RPC], BF16)
            a2a_out2 = dram.tile([NCORES, HD, RPC], BF16)
            # ---- PE warmup: flip HAM to K=8/8 before real matmuls ----
            with tc.tile_pool(name="psw", bufs=1, space="PSUM") as psw:
                ps_w = psw.tile([128, 128], F32, name="ps_w")
                for _ in range(32):
                    pe(nc.tensor.matmul(ps_w[:], wrm[:], wrm[:],
                                        start=True, stop=True))
                close_unit()

            def rope(dst, ps, c0, rope_pool):
                """dst[128,512] bf16 = ps*cos + swap_halves(ps)*ssin."""
                ra = rope_pool.tile([128, 512], BF16, name="ra", tag="ra",
                                    bufs=2)
                rb = rope_pool.tile([128, 512], BF16, name="rb", tag="rb",
                                    bufs=2)
                ve(nc.vector.tensor_tensor(
                    ra[:], ps[:], cos_sb[:, c0:c0 + 512],
                    mybir.AluOpType.mult))
                ve(nc.vector.tensor_tensor(
                    rb[0:64, :], ps[64:128, :], sin_sb[0:64, c0:c0 + 512],
                    mybir.AluOpType.mult))
                ve(nc.vector.tensor_tensor(
                    rb[64:128, :], ps[0:64, :], sin_sb[64:128, c0:c0 + 512],
                    mybir.AluOpType.mult))
                ve(nc.vector.tensor_tensor(dst, ra[:], rb[:],
                                           mybir.AluOpType.add))
                close_vunit()

            with (
                tc.tile_pool(name="ps2", bufs=1, space="PSUM") as ps2,
            ):
                # ====== fused projection phase (single x stream) =========
                fstack = contextlib.ExitStack()
                xtiles = fstack.enter_context(
                    tc.tile_pool(name="xtiles", bufs=1))
                ropeF = fstack.enter_context(tc.tile_pool(name="ropeF",
                                                          bufs=1))
                kvout = fstack.enter_context(tc.tile_pool(name="kvout",
                                                          bufs=1))
                qstr = fstack.enter_context(tc.tile_pool(name="qstr", bufs=1))


                kTh = kvout.tile([128, S], BF16)
                vTh = kvout.tile([128, S], BF16)
                vh = kvout.tile([128, S // 128, HD], BF16)
                q_str = qstr.tile([128, HPC, POS], BF16)

                for t in range(PT_N):
                    c0 = (t * 512) % S
                    x_t = xtiles.tile([128, KT, 512], BF16, name="x_t",
                                      tag="x", bufs=2)
                    if t == 0:
                        # 2-ktile chunks on two rings: the first matmuls
                        # start after ~0.25MB and the rings fill in parallel
                        for k8 in range(8):
                            ceng = nc.sync if k8 % 2 == 0 else nc.scalar
                            ceng.dma_start(
                                x_t[:, k8 * 2:(k8 + 1) * 2, :],
                                xT.ap()[t, :, k8 * 2:(k8 + 1) * 2, :])
                        nc.scalar.dma_start(cos_sb[:], cosT.ap())
                        nc.scalar.dma_start(sin_sb[:], ssinT.ap())
                        nc.scalar.dma_start(qsel_sb[:], qsel.ap())
                        nc.scalar.dma_start(wq_sb[:], wq.ap())
                    else:
                        eng = nc.sync if t % 2 == 0 else nc.scalar
                        eng.dma_start(x_t[:], xT.ap()[t])
                    if t < HPT:
                        ps_k2 = ps2.tile([128, 2, 512], F32, name="ps_k",
                                         tag="stp", bufs=2)
                        ps_k = ps_k2[:, 0, :]
                        for k in range(KT):
                            pe(nc.tensor.matmul(ps_k[:], wk_sb[:, k, :],
                                                x_t[:, k, :], start=(k == 0),
                                                stop=(k == KT - 1)))
                        close_unit()
                        rope(kTh[:, t * 512:(t + 1) * 512], ps_k, c0, ropeF)
                        ps_v2 = ps2.tile([128, 2, 512], F32, name="ps_v",
                                         tag="stp", bufs=2)
                        ps_v = ps_v2[:, 0, :]
                        for k in range(KT):
                            pe(nc.tensor.matmul(ps_v[:], wv_sb[:, k, :],
                                                x_t[:, k, :], start=(k == 0),
                                                stop=(k == KT - 1)))
                        close_unit()
                        nc.scalar.copy(vTh[:, t * 512:(t + 1) * 512], ps_v[:])
                    for hh in range(HPC):
                        ps_q2 = ps2.tile([128, 2, 512], F32, name="ps_q",
                                         tag="stp", bufs=2)
                        ps_q = ps_q2[:, 0, :]
                        for k in range(KT):
                            pe(nc.tensor.matmul(
                                ps_q[:], wq_sb[:, k, hh * 128:(hh + 1) * 128],
                                x_t[:, k, :], start=(k == 0),
                                stop=(k == KT - 1)))
                        close_unit()
                        rope(q_str[:, hh, t * 512:(t + 1) * 512], ps_q,
                             c0, ropeF)
                    if t == 1:
                        # first k half staged for exchange as soon as ready
                        nc.gpsimd.dma_start(exch_in[0], kTh[:, 0:1024])
                    if t == 2:
                        nc.scalar.dma_start_transpose(vh[:, 0:8, :],
                                                      vTh[:, 0:1024])
                        nc.gpsimd.dma_start(exch_in[2], vh[:, 0:8, :])
                    if t == HPT - 1:
                        # second halves, then ONE AllGather for all of k/v
                        # (a single CC op: one barrier-gated start, one
                        # trigger, no inter-op ncfw lag)
                        nc.gpsimd.dma_start(exch_in[1], kTh[:, 1024:2048])
                        nc.scalar.dma_start_transpose(vh[:, 8:16, :],
                                                      vTh[:, 1024:2048])
                        nc.gpsimd.dma_start(exch_in[3], vh[:, 8:16, :])
                        nc.gpsimd.collective_compute(
                            "AllGather", mybir.AluOpType.bypass,
                            replica_groups=[[0, 1], [2, 3], [4, 5], [6, 7]],
                            ins=[exch_in.opt()], outs=[exch_out.opt()])
                # load the gathered k/v on the quiet gpsimd ring, ordered
                # by when attention needs them
                nc.gpsimd.dma_start(kT_all[:, 0:1024], exch_out[0, 0])
                nc.gpsimd.dma_start(kT_all[:, 2048:3072], exch_out[1, 0])
                nc.gpsimd.dma_start(v_all[:, 0:8, :], exch_out[0, 2])
                nc.gpsimd.dma_start(v_all[:, 16:24, :], exch_out[1, 2])
                nc.gpsimd.dma_start(kT_all[:, 1024:2048], exch_out[0, 1])
                nc.gpsimd.dma_start(kT_all[:, 3072:4096], exch_out[1, 1])
                nc.gpsimd.dma_start(v_all[:, 8:16, :], exch_out[0, 3])
                kld = nc.gpsimd.dma_start(v_all[:, 24:32, :],
                                          exch_out[1, 3])

                # wo prefetch: transfers run during attention, but only
                # after the kv-exchange collective is off the wire
                for k4 in range(2):
                    nc.scalar.dma_start(
                        woe_sb[:, k4 * 4:(k4 + 1) * 4, :],
                        wo.ap()[0, :, k4 * 4:(k4 + 1) * 4, :])
                for k4 in range(2):
                    wd = nc.sync.dma_start(
                        woo_sb[:, k4 * 4:(k4 + 1) * 4, :],
                        wo.ap()[1, :, k4 * 4:(k4 + 1) * 4, :])
                    add_dep_helper(wd.ins, kld.ins, True)

                # rearrange q into global batch order (mask select)
                for hh in range(HPC):
                    for gb in range(B):
                        for c in range(4):
                            lo = gb * S + c * 512
                            alo = (1 - gb) * S + c * 512
                            nc.vector.select(
                                q_all[:, hh, lo:lo + 512], qsel_sb[:],
                                q_str[:, hh, alo:alo + 512],
                                q_str[:, hh, lo:lo + 512])
                        close_vunit()
                fstack.close()
                astack = contextlib.ExitStack()
                att = astack.enter_context(tc.tile_pool(name="att", bufs=1))

                # ====== attention: flattened cross-unit pipeline =========
                # Consumers (PV + denominator matmuls) lag the scores stream
                # by LAG pairs so the exp -> pair-add chain is always hidden,
                # across unit boundaries too.
                LAG = 4

                def emit_consumers(e):
                    u = e["u"]
                    pe(nc.tensor.matmul(
                        u["o_ps"][:, e["c00"]:512],
                        v_all[:, u["voff"] + e["kt0"], :],
                        e["pt"][:, 0, e["c00"]:512], start=(e["kt0"] == 0),
                        stop=False))
                    pe(nc.tensor.matmul(
                        u["o_ps"][:, e["c01"]:512],
                        v_all[:, u["voff"] + e["kt1"], :],
                        e["pt"][:, 1, e["c01"]:512], start=False,
                        stop=(e["kt1"] == u["nkt"] - 1)))
                    pe(nc.tensor.matmul(
                        u["sum_ps"][:, e["c00"]:512], ones_sb[:],
                        e["padd"][:, e["c00"]:512], start=(e["pr"] == 0),
                        stop=(e["pr"] == u["nkt"] // 2 - 1)))
                    if e["pr"] == u["nkt"] // 2 - 1:
                        # unit tail: normalize and stage for the AllToAll
                        recip = att.tile([128, 512], F32, name="recip",
                                         tag="recip", bufs=2)
                        ve(nc.vector.reciprocal_approx_fast(recip[:],
                                                            u["sum_ps"][:]))
                        oT_sb = att.tile([128, 512], BF16, name="oT_sb",
                                         tag="osb", bufs=2)
                        ve(nc.vector.scalar_tensor_tensor(
                            oT_sb[:], u["o_ps"][:], 1.0, recip[:],
                            mybir.AluOpType.mult, mybir.AluOpType.mult))
                        close_vunit()
                        hh, b, qsb = u["key"]
                        a2a_in = a2a_in1 if hh == 0 else a2a_in2
                        nc.gpsimd.dma_start(a2a_in[b * 4 + qsb, :, :],
                                            oT_sb[:])
                        if u["key"] == (0, 1, 3):
                            nc.gpsimd.collective_compute(
                                "AllToAll", mybir.AluOpType.bypass,
                                replica_groups=[list(range(NCORES))],
                                ins=[a2a_in1.opt()], outs=[a2a_out1.opt()])
                        elif u["key"] == (1, 1, 3):
                            nc.gpsimd.collective_compute(
                                "AllToAll", mybir.AluOpType.bypass,
                                replica_groups=[list(range(NCORES))],
                                ins=[a2a_in2.opt()], outs=[a2a_out2.opt()])

                inflight = []
                for hh in range(HPC):
                    for b in range(B):
                        for qsb in range(4):
                            qT = q_all[:, hh, b * S:(b + 1) * S]
                            kTb = kT_all[:, b * S:(b + 1) * S]
                            qs = qsb * 512
                            nkt = 4 * qsb + 4
                            u = {"key": (hh, b, qsb), "nkt": nkt,
                                 "voff": b * (S // 128),
                                 "o_ps": ps2.tile([128, 512], F32,
                                                  name="o_ps", tag="ops",
                                                  bufs=2),
                                 "sum_ps": ps2.tile([128, 512], F32,
                                                    name="sum_ps", tag="sums",
                                                    bufs=2)}
                            for pr in range(nkt // 2):
                                kt0, kt1 = 2 * pr, 2 * pr + 1
                                jj0, jj1 = kt0 - 4 * qsb, kt1 - 4 * qsb
                                c00 = 0 if jj0 < 0 else jj0 * 128
                                c01 = 0 if jj1 < 0 else jj1 * 128
                                st = ps2.tile([128, 2, 512], F32, name="st",
                                              tag="stp", bufs=2)
                                pe(nc.tensor.matmul(
                                    st[:, 0, c00:512],
                                    kTb[:, kt0 * 128:(kt0 + 1) * 128],
                                    qT[:, qs + c00:qs + 512], start=True,
                                    stop=True))
                                sc2 = pe(nc.tensor.matmul(
                                    st[:, 1, c01:512],
                                    kTb[:, kt1 * 128:(kt1 + 1) * 128],
                                    qT[:, qs + c01:qs + 512], start=True,
                                    stop=True))
                                pt_sb = att.tile([128, 2, 512], BF16,
                                                 name="pt_sb", tag="ptp",
                                                 bufs=6)
                                nc.scalar.activation(
                                    pt_sb[:, :, c00:512], st[:, :, c00:512],
                                    mybir.ActivationFunctionType.Exp,
                                    scale=SCALE)
                                if jj0 >= 0:
                                    ve(nc.vector.tensor_tensor(
                                        pt_sb[:, 0, jj0 * 128:(jj0 + 1) * 128],
                                        pt_sb[:, 0, jj0 * 128:(jj0 + 1) * 128],
                                        triT[:], mybir.AluOpType.mult))
                                    ve(nc.vector.tensor_tensor(
                                        pt_sb[:, 1, jj1 * 128:(jj1 + 1) * 128],
                                        pt_sb[:, 1, jj1 * 128:(jj1 + 1) * 128],
                                        triT[:], mybir.AluOpType.mult))
                                # pair-sum for the softmax denominator
                                padd = att.tile([128, 512], BF16,
                                                name="padd", tag="padd",
                                                bufs=6)
                                if jj0 < 0:
                                    ve(nc.vector.tensor_tensor(
                                        padd[:], pt_sb[:, 0, :],
                                        pt_sb[:, 1, :], mybir.AluOpType.add))
                                else:
                                    ve(nc.vector.tensor_copy(
                                        padd[:, c00:c01],
                                        pt_sb[:, 0, c00:c01]))
                                    ve(nc.vector.tensor_tensor(
                                        padd[:, c01:512],
                                        pt_sb[:, 0, c01:512],
                                        pt_sb[:, 1, c01:512],
                                        mybir.AluOpType.add))
                                if pr == nkt // 2 - 1:
                                    # unit boundary for the PE chain
                                    unit_last[0] = sc2
                                    unit_first[0] = None
                                inflight.append(
                                    {"u": u, "pr": pr, "kt0": kt0,
                                     "kt1": kt1, "c00": c00, "c01": c01,
                                     "pt": pt_sb, "padd": padd})
                                while len(inflight) > LAG:
                                    emit_consumers(inflight.pop(0))
                while inflight:
                    emit_consumers(inflight.pop(0))

                # ====== o_proj (contraction split by head-half) ==============
                with tc.tile_pool(name="proj", bufs=1) as proj:
                    at1_sb = proj.tile([128, NCORES, RPC], BF16)
                    at2_sb = proj.tile([128, NCORES, RPC], BF16)
                    for r in range(NCORES):
                        nc.sync.dma_start(at1_sb[:, r, :], a2a_out1[r, :, :])
                    s1_sb = proj.tile([128, 16, 512], BF16)

                    def part1_unit(ti):
                        mp, nn = ti // 4, ti % 4
                        ps_a = ps2.tile([128, 512], F32, name="ps_a",
                                         tag="ops", bufs=2)
                        for r in range(NCORES):
                            pe(nc.tensor.matmul(
                                ps_a[:],
                                at1_sb[:, r, mp * 128:(mp + 1) * 128],
                                woe_sb[:, r, nn * 512:(nn + 1) * 512],
                                start=(r == 0), stop=(r == NCORES - 1)))
                        close_unit()
                        ve(nc.vector.tensor_copy(s1_sb[:, ti, :], ps_a[:]))
                        close_vunit()

                    for ti in range(16):
                        part1_unit(ti)

                    # part 2 (h1 contraction) + output
                    for r in range(NCORES):
                        nc.scalar.dma_start(at2_sb[:, r, :],
                                            a2a_out2[r, :, :])
                    for nn in range(H // 512):
                        for mp in range(RPC // 128):
                            ti = mp * 4 + nn
                            ps_b = ps2.tile([128, 512], F32, name="ps_b",
                                             tag="ops", bufs=2)
                            for r in range(NCORES):
                                pe(nc.tensor.matmul(
                                    ps_b[:],
                                    at2_sb[:, r, mp * 128:(mp + 1) * 128],
                                    woo_sb[:, r, nn * 512:(nn + 1) * 512],
                                    start=(r == 0), stop=(r == NCORES - 1)))
                            close_unit()
                            ev = proj.tile([128, 512], BF16, name="ev",
                                           tag="ev", bufs=4)
                            ve(nc.vector.scalar_tensor_tensor(
                                ev[:], ps_b[:], 1.0, s1_sb[:, ti, :],
                                mybir.AluOpType.mult, mybir.AluOpType.add))
                            close_vunit()
                            oeng = nc.sync if ti % 2 == 0 else nc.scalar
                            oeng.dma_start(
                                out.ap()[mp * 128:(mp + 1) * 128,
                                         nn * 512:(nn + 1) * 512], ev[:])
                astack.close()

    nc.compile()
    return nc


def _get_nc():
    if "nc" not in _CACHE:
        _CACHE["nc"] = _build()
    return _CACHE["nc"]


def _prep_inputs(x, cos, sin, wq, wk, wv, wo):
    bf = ml_dtypes.bfloat16
    xf = np.asarray(x, np.float32).reshape(POS, H)
    # [PT_N, 128, KT, 512]: xTt[pt,p,k,j] = x[pt*512+j, k*128+p]
    # (partition-major: each tile loads as one contiguous 2D DMA)
    xT = np.ascontiguousarray(
        xf.reshape(PT_N, 512, KT, 128).transpose(0, 3, 2, 1)).astype(bf)
    cosT = np.ascontiguousarray(np.asarray(cos, np.float32).T).astype(bf)
    sinT = np.asarray(sin, np.float32).T.copy()
    sinT[0:64, :] = -sinT[0:64, :]
    sinT = np.ascontiguousarray(sinT).astype(bf)
    # wo split even/odd contraction tiles, partition-major:
    # wo_b[e, p, kk, m] = wo[(2*kk+e)*128 + p, m]
    wo_r = np.asarray(wo, np.float32).reshape(KT, 128, H)
    wo_b = np.ascontiguousarray(
        np.stack([wo_r[0::2], wo_r[1::2]], axis=0).transpose(0, 2, 1, 3)
    ).astype(bf)
    wq = np.asarray(wq, np.float32)
    wk = np.asarray(wk, np.float32)
    wv = np.asarray(wv, np.float32)
    sel0 = np.zeros((128, 512), np.uint8)
    sel1 = np.ones((128, 512), np.uint8)

    in_maps = []
    for i in range(NCORES):
        kv = i // 2
        half = i % 2
        xp = np.ascontiguousarray(np.concatenate(
            [xT[half * HPT:(half + 1) * HPT],
             xT[(1 - half) * HPT:(2 - half) * HPT]], axis=0))
        in_maps.append({
            "xT": xp,
            "wq": np.ascontiguousarray(
                wq[:, i * HPC * HD:(i + 1) * HPC * HD].reshape(
                    KT, 128, HPC * HD).transpose(1, 0, 2)).astype(bf),
            "wk": np.ascontiguousarray(
                wk[:, kv * HD:(kv + 1) * HD].reshape(
                    KT, 128, HD).transpose(1, 0, 2)).astype(bf),
            "wv": np.ascontiguousarray(
                wv[:, kv * HD:(kv + 1) * HD].reshape(
                    KT, 128, HD).transpose(1, 0, 2)).astype(bf),
            "cosT": cosT,
            "ssinT": sinT,
            "qsel": sel1 if half else sel0,
            "wo": wo_b,
        })
    return in_maps


def kernel(x, cos, sin, wq, wk, wv, wo, _trace=False):
    nc = _get_nc()
    in_maps = _prep_inputs(x, cos, sin, wq, wk, wv, wo)
    res = run_bass_kernel_spmd(nc, in_maps, core_ids=list(range(NCORES)),
                               trace=_trace)
    rows = np.concatenate([np.asarray(res.results[i]["out"], np.float32)
                           for i in range(NCORES)], axis=0)
    out = rows.reshape(B, S, H)
    if _trace:
        _CACHE["last_exec_time_ns"] = res.exec_time_ns
        _CACHE["last_results"] = res
    return out



# revision 23
# speedup vs baseline: 2.1315x; 2.1315x over previous
"""Distributed GQA attention (B=2,S=2048,H=2048,NH=16,NKV=4,HD=128) on 8 TRN2 cores.

Strategy: tensor-parallel over heads (2 Q heads + 1 KV head per core).
Each core streams x once in a per-core order (its kv half first): the
first 4 pos-tiles produce K^T and V^T for its half (512-wide matmuls; V is
fixed up by a DMA transpose) plus Q; the rest produce Q only. K/V halves
are exchanged with a pairwise AllGather (even rank = batch 0, so the
gathered layout is globally batch-ordered); Q is rearranged to global
batch order with mask selects driven by a per-core 0/1 input. Causal
flash attention runs in scores-transposed layout with kt-paired exp
instructions and pair-summed softmax denominators. An AllToAll per
head-half switches to sequence-parallel o_proj.
"""

import contextlib
import math

import numpy as np
import ml_dtypes

import concourse.bass as bass
import concourse.mybir as mybir
import concourse.tile as tile
from concourse.tile import add_dep_helper
from concourse import bacc
from concourse.bass_utils import run_bass_kernel_spmd
from concourse.masks import make_identity

BF16 = mybir.dt.bfloat16
F32 = mybir.dt.float32

B, S, H = 2, 2048, 2048
NH, NKV, HD = 16, 4, 128
NCORES = 8
HPC = NH // NCORES          # q heads per core = 2
POS = B * S                 # 4096 flattened rows
RPC = POS // NCORES         # output rows per core = 512
KT = H // 128               # 16 contraction tiles for projections
PT_N = POS // 512           # 8 pos-tiles of 512
HPT = PT_N // 2             # pos-tiles in my kv half = 4
SCALE = 1.0 / math.sqrt(HD)

_CACHE = {}


def _build():
    nc = bacc.Bacc("TRN2", target_bir_lowering=False, debug=False,
                   num_devices=NCORES)

    # all parameters are host-prepacked partition-major so every DMA is a
    # contiguous-per-partition 2D block (cheap DIRECT2D issue, few
    # descriptors)
    xT = nc.declare_dram_parameter("xT", [PT_N, 128, KT, 512], BF16,
                                   isOutput=False)
    wq = nc.declare_dram_parameter("wq", [128, KT, HPC * HD], BF16,
                                   isOutput=False)
    wk = nc.declare_dram_parameter("wk", [128, KT, HD], BF16, isOutput=False)
    wv = nc.declare_dram_parameter("wv", [128, KT, HD], BF16, isOutput=False)
    cosT = nc.declare_dram_parameter("cosT", [HD, S], BF16, isOutput=False)
    ssinT = nc.declare_dram_parameter("ssinT", [HD, S], BF16, isOutput=False)
    qsel = nc.declare_dram_parameter("qsel", [128, 512], mybir.dt.uint8,
                                     isOutput=False)
    wo = nc.declare_dram_parameter("wo", [2, 128, KT // 2, H], BF16,
                                   isOutput=False)
    out = nc.declare_dram_parameter("out", [RPC, H], BF16,
                                    isOutput=True)

    unit_last = [None]
    unit_first = [None]
    unit_latest = [None]

    def pe(mm):
        # chain PE work at unit granularity: the first matmul of each unit
        # depends on the last matmul of the previous unit; within a unit the
        # scheduler is free to pipeline.
        if unit_first[0] is None:
            unit_first[0] = mm
            if unit_last[0] is not None:
                add_dep_helper(mm.ins, unit_last[0].ins, False)
        unit_latest[0] = mm
        return mm

    def close_unit():
        unit_last[0] = unit_latest[0]
        unit_first[0] = None

    vunit_last = [None]
    vunit_first = [None]
    vunit_latest = [None]

    def ve(op):
        # same unit-granularity chain for the vector queue, so late critical
        # ops (stt/recip) aren't scheduled behind later units' vector work
        if vunit_first[0] is None:
            vunit_first[0] = op
            if vunit_last[0] is not None:
                add_dep_helper(op.ins, vunit_last[0].ins, False)
        vunit_latest[0] = op
        return op

    def close_vunit():
        vunit_last[0] = vunit_latest[0]
        vunit_first[0] = None

    with tile.TileContext(nc) as tc:
        with (
            tc.tile_pool(name="const", bufs=1) as const,
            tc.tile_pool(name="wpool", bufs=1) as wpool,
            tc.tile_pool(name="qkv", bufs=1) as qkv,
            tc.tile_pool(name="dram", bufs=1, space="DRAM") as dram,
        ):
            # warmup scratch first: one cheap vector memset unblocks the
            # HAM-warmup matmuls immediately
            wrm = const.tile([128, 128], BF16)
            nc.vector.memset(wrm, 0.5)

            # kv-proj weights first on gpsimd: needed by the very first
            # matmuls
            wk_sb = wpool.tile([128, KT, HD], BF16)
            wv_sb = wpool.tile([128, KT, HD], BF16)
            nc.gpsimd.dma_start(wk_sb[:], wk.ap())
            nc.gpsimd.dma_start(wv_sb[:], wv.ap())

            # ---- constants / weights resident in SBUF ----
            ident = const.tile([128, 128], BF16)
            make_identity(nc, ident)
            # upper-triangular (incl diag) mask: valid where kpos <= q
            triT = const.tile([128, 128], BF16)
            nc.gpsimd.memset(triT, 1.0)
            nc.gpsimd.affine_select(
                out=triT, in_=triT, compare_op=mybir.AluOpType.is_ge,
                fill=0.0, base=0, pattern=[[1, 128]], channel_multiplier=-1,
            )  # keep 1.0 where (c - p) >= 0, i.e. kpos <= q
            ones_sb = const.tile([128, 128], BF16)
            nc.gpsimd.memset(ones_sb, 1.0)

            cos_sb = const.tile([128, S], BF16)
            sin_sb = const.tile([128, S], BF16)
            qsel_sb = const.tile([128, 512], mybir.dt.uint8)

            wq_sb = wpool.tile([128, KT, HPC * HD], BF16)
            woe_sb = wpool.tile([128, KT // 2, H], BF16)
            woo_sb = wpool.tile([128, KT // 2, H], BF16)

            # persistent q/k/v for both batches, global batch order (bf16)
            q_all = qkv.tile([128, HPC, POS], BF16)
            kT_all = qkv.tile([128, POS], BF16)
            v_all = qkv.tile([128, POS // 128, HD], BF16)

            # single merged kv exchange:
            # [0]=kT half a, [1]=kT half b, [2]=v half a, [3]=v half b
            exch_in = dram.tile([4, 128, 1024], BF16)
            exch_out = dram.tile([2, 4, 128, 1024], BF16)
            a2a_in1 = dram.tile([NCORES, HD, RPC], BF16)
            a2a_out1 = dram.tile([NCORES, HD, RPC], BF16)
            a2a_in2 = dram.tile([NCORES, HD, RPC], BF16)
            a2a_out2 = dram.tile([NCORES, HD, RPC], BF16)
            # ---- PE warmup: flip HAM to K=8/8 before real matmuls ----
            with tc.tile_pool(name="psw", bufs=1, space="PSUM") as psw:
                ps_w = psw.tile([128, 128], F32, name="ps_w")
                for _ in range(32):
                    pe(nc.tensor.matmul(ps_w[:], wrm[:], wrm[:],
                                        start=True, stop=True))
                close_unit()

            def rope(dst, ps, c0, rope_pool):
                """dst[128,512] bf16 = ps*cos + swap_halves(ps)*ssin."""
                ra = rope_pool.tile([128, 512], BF16, name="ra", tag="ra",
                                    bufs=2)
                rb = rope_pool.tile([128, 512], BF16, name="rb", tag="rb",
                                    bufs=2)
                ve(nc.vector.tensor_tensor(
                    ra[:], ps[:], cos_sb[:, c0:c0 + 512],
                    mybir.AluOpType.mult))
                ve(nc.vector.tensor_tensor(
                    rb[0:64, :], ps[64:128, :], sin_sb[0:64, c0:c0 + 512],
                    mybir.AluOpType.mult))
                ve(nc.vector.tensor_tensor(
                    rb[64:128, :], ps[0:64, :], sin_sb[64:128, c0:c0 + 512],
                    mybir.AluOpType.mult))
                ve(nc.vector.tensor_tensor(dst, ra[:], rb[:],
                                           mybir.AluOpType.add))
                close_vunit()

            with (
                tc.tile_pool(name="ps2", bufs=1, space="PSUM") as ps2,
            ):
                # ====== fused projection phase (single x stream) =========
                fstack = contextlib.ExitStack()
                xtiles = fstack.enter_context(
                    tc.tile_pool(name="xtiles", bufs=1))
                ropeF = fstack.enter_context(tc.tile_pool(name="ropeF",
                                                          bufs=1))
                kvout = fstack.enter_context(tc.tile_pool(name="kvout",
                                                          bufs=1))
                qstr = fstack.enter_context(tc.tile_pool(name="qstr", bufs=1))


                kTh = kvout.tile([128, S], BF16)
                vTh = kvout.tile([128, S], BF16)
                vh = kvout.tile([128, S // 128, HD], BF16)
                q_str = qstr.tile([128, HPC, POS], BF16)

                for t in range(PT_N):
                    c0 = (t * 512) % S
                    x_t = xtiles.tile([128, KT, 512], BF16, name="x_t",
                                      tag="x", bufs=2)
                    if t == 0:
                        # 2-ktile chunks on two rings: the first matmuls
                        # start after ~0.25MB and the rings fill in parallel
                        for k8 in range(8):
                            ceng = nc.sync if k8 % 2 == 0 else nc.scalar
                            ceng.dma_start(
                                x_t[:, k8 * 2:(k8 + 1) * 2, :],
                                xT.ap()[t, :, k8 * 2:(k8 + 1) * 2, :])
                        nc.scalar.dma_start(cos_sb[:], cosT.ap())
                        nc.scalar.dma_start(sin_sb[:], ssinT.ap())
                        nc.scalar.dma_start(qsel_sb[:], qsel.ap())
                        nc.scalar.dma_start(wq_sb[:], wq.ap())
                    else:
                        eng = nc.sync if t % 2 == 0 else nc.scalar
                        eng.dma_start(x_t[:], xT.ap()[t])
                    if t < HPT:
                        ps_k2 = ps2.tile([128, 2, 512], F32, name="ps_k",
                                         tag="stp", bufs=2)
                        ps_k = ps_k2[:, 0, :]
                        for k in range(KT):
                            pe(nc.tensor.matmul(ps_k[:], wk_sb[:, k, :],
                                                x_t[:, k, :], start=(k == 0),
                                                stop=(k == KT - 1)))
                        close_unit()
                        rope(kTh[:, t * 512:(t + 1) * 512], ps_k, c0, ropeF)
                        ps_v2 = ps2.tile([128, 2, 512], F32, name="ps_v",
                                         tag="stp", bufs=2)
                        ps_v = ps_v2[:, 0, :]
                        for k in range(KT):
                            pe(nc.tensor.matmul(ps_v[:], wv_sb[:, k, :],
                                                x_t[:, k, :], start=(k == 0),
                                                stop=(k == KT - 1)))
                        close_unit()
                        nc.scalar.copy(vTh[:, t * 512:(t + 1) * 512], ps_v[:])
                    for hh in range(HPC):
                        ps_q2 = ps2.tile([128, 2, 512], F32, name="ps_q",
                                         tag="stp", bufs=2)
                        ps_q = ps_q2[:, 0, :]
                        for k in range(KT):
                            pe(nc.tensor.matmul(
                                ps_q[:], wq_sb[:, k, hh * 128:(hh + 1) * 128],
                                x_t[:, k, :], start=(k == 0),
                                stop=(k == KT - 1)))
                        close_unit()
                        rope(q_str[:, hh, t * 512:(t + 1) * 512], ps_q,
                             c0, ropeF)
                    if t == 1:
                        # first k half staged for exchange as soon as ready
                        nc.gpsimd.dma_start(exch_in[0], kTh[:, 0:1024])
                    if t == 2:
                        nc.scalar.dma_start_transpose(vh[:, 0:8, :],
                                                      vTh[:, 0:1024])
                        nc.gpsimd.dma_start(exch_in[2], vh[:, 0:8, :])
                    if t == HPT - 1:
                        # second halves, then ONE AllGather for all of k/v
                        # (a single CC op: one barrier-gated start, one
                        # trigger, no inter-op ncfw lag)
                        nc.gpsimd.dma_start(exch_in[1], kTh[:, 1024:2048])
                        nc.scalar.dma_start_transpose(vh[:, 8:16, :],
                                                      vTh[:, 1024:2048])
                        nc.gpsimd.dma_start(exch_in[3], vh[:, 8:16, :])
                        nc.gpsimd.collective_compute(
                            "AllGather", mybir.AluOpType.bypass,
                            replica_groups=[[0, 1], [2, 3], [4, 5], [6, 7]],
                            ins=[exch_in.opt()], outs=[exch_out.opt()])
                # load the gathered k/v on the quiet gpsimd ring: all
                # batch-0 pieces first (attention iterates b=0 first)
                nc.gpsimd.dma_start(kT_all[:, 0:1024], exch_out[0, 0])
                nc.gpsimd.dma_start(kT_all[:, 1024:2048], exch_out[0, 1])
                nc.gpsimd.dma_start(v_all[:, 0:8, :], exch_out[0, 2])
                nc.gpsimd.dma_start(v_all[:, 8:16, :], exch_out[0, 3])
                nc.gpsimd.dma_start(kT_all[:, 2048:3072], exch_out[1, 0])
                nc.gpsimd.dma_start(kT_all[:, 3072:4096], exch_out[1, 1])
                nc.gpsimd.dma_start(v_all[:, 16:24, :], exch_out[1, 2])
                kld = nc.gpsimd.dma_start(v_all[:, 24:32, :],
                                          exch_out[1, 3])

                # wo prefetch: transfers run during attention, but only
                # after the kv-exchange collective is off the wire
                for k4 in range(2):
                    nc.scalar.dma_start(
                        woe_sb[:, k4 * 4:(k4 + 1) * 4, :],
                        wo.ap()[0, :, k4 * 4:(k4 + 1) * 4, :])
                for k4 in range(2):
                    wd = nc.sync.dma_start(
                        woo_sb[:, k4 * 4:(k4 + 1) * 4, :],
                        wo.ap()[1, :, k4 * 4:(k4 + 1) * 4, :])
                    add_dep_helper(wd.ins, kld.ins, True)

                # rearrange q into global batch order (mask select)
                for hh in range(HPC):
                    for gb in range(B):
                        for c in range(4):
                            lo = gb * S + c * 512
                            alo = (1 - gb) * S + c * 512
                            nc.vector.select(
                                q_all[:, hh, lo:lo + 512], qsel_sb[:],
                                q_str[:, hh, alo:alo + 512],
                                q_str[:, hh, lo:lo + 512])
                        close_vunit()
                fstack.close()
                astack = contextlib.ExitStack()
                att = astack.enter_context(tc.tile_pool(name="att", bufs=1))

                # ====== attention: flattened cross-unit pipeline =========
                # Consumers (PV + denominator matmuls) lag the scores stream
                # by LAG pairs so the exp -> pair-add chain is always hidden,
                # across unit boundaries too.
                LAG = 4

                def emit_consumers(e):
                    u = e["u"]
                    pe(nc.tensor.matmul(
                        u["o_ps"][:, e["c00"]:512],
                        v_all[:, u["voff"] + e["kt0"], :],
                        e["pt"][:, 0, e["c00"]:512], start=(e["kt0"] == 0),
                        stop=False))
                    pe(nc.tensor.matmul(
                        u["o_ps"][:, e["c01"]:512],
                        v_all[:, u["voff"] + e["kt1"], :],
                        e["pt"][:, 1, e["c01"]:512], start=False,
                        stop=(e["kt1"] == u["nkt"] - 1)))
                    pe(nc.tensor.matmul(
                        u["sum_ps"][:, e["c00"]:512], ones_sb[:],
                        e["padd"][:, e["c00"]:512], start=(e["pr"] == 0),
                        stop=(e["pr"] == u["nkt"] // 2 - 1)))
                    if e["pr"] == u["nkt"] // 2 - 1:
                        # unit tail: normalize and stage for the AllToAll
                        recip = att.tile([128, 512], F32, name="recip",
                                         tag="recip", bufs=2)
                        ve(nc.vector.reciprocal_approx_fast(recip[:],
                                                            u["sum_ps"][:]))
                        oT_sb = att.tile([128, 512], BF16, name="oT_sb",
                                         tag="osb", bufs=2)
                        ve(nc.vector.scalar_tensor_tensor(
                            oT_sb[:], u["o_ps"][:], 1.0, recip[:],
                            mybir.AluOpType.mult, mybir.AluOpType.mult))
                        close_vunit()
                        hh, b, qsb = u["key"]
                        a2a_in = a2a_in1 if hh == 0 else a2a_in2
                        nc.gpsimd.dma_start(a2a_in[b * 4 + qsb, :, :],
                                            oT_sb[:])
                        if u["key"] == (0, 1, 3):
                            nc.gpsimd.collective_compute(
                                "AllToAll", mybir.AluOpType.bypass,
                                replica_groups=[list(range(NCORES))],
                                ins=[a2a_in1.opt()], outs=[a2a_out1.opt()])
                        elif u["key"] == (1, 1, 3):
                            nc.gpsimd.collective_compute(
                                "AllToAll", mybir.AluOpType.bypass,
                                replica_groups=[list(range(NCORES))],
                                ins=[a2a_in2.opt()], outs=[a2a_out2.opt()])

                inflight = []
                for hh in range(HPC):
                    for b in range(B):
                        for qsb in range(4):
                            qT = q_all[:, hh, b * S:(b + 1) * S]
                            kTb = kT_all[:, b * S:(b + 1) * S]
                            qs = qsb * 512
                            nkt = 4 * qsb + 4
                            u = {"key": (hh, b, qsb), "nkt": nkt,
                                 "voff": b * (S // 128),
                                 "o_ps": ps2.tile([128, 512], F32,
                                                  name="o_ps", tag="ops",
                                                  bufs=2),
                                 "sum_ps": ps2.tile([128, 512], F32,
                                                    name="sum_ps", tag="sums",
                                                    bufs=2)}
                            for pr in range(nkt // 2):
                                kt0, kt1 = 2 * pr, 2 * pr + 1
                                jj0, jj1 = kt0 - 4 * qsb, kt1 - 4 * qsb
                                c00 = 0 if jj0 < 0 else jj0 * 128
                                c01 = 0 if jj1 < 0 else jj1 * 128
                                st = ps2.tile([128, 2, 512], F32, name="st",
                                              tag="stp", bufs=2)
                                pe(nc.tensor.matmul(
                                    st[:, 0, c00:512],
                                    kTb[:, kt0 * 128:(kt0 + 1) * 128],
                                    qT[:, qs + c00:qs + 512], start=True,
                                    stop=True))
                                sc2 = pe(nc.tensor.matmul(
                                    st[:, 1, c01:512],
                                    kTb[:, kt1 * 128:(kt1 + 1) * 128],
                                    qT[:, qs + c01:qs + 512], start=True,
                                    stop=True))
                                pt_sb = att.tile([128, 2, 512], BF16,
                                                 name="pt_sb", tag="ptp",
                                                 bufs=6)
                                nc.scalar.activation(
                                    pt_sb[:, :, c00:512], st[:, :, c00:512],
                                    mybir.ActivationFunctionType.Exp,
                                    scale=SCALE)
                                if jj0 >= 0:
                                    ve(nc.vector.tensor_tensor(
                                        pt_sb[:, 0, jj0 * 128:(jj0 + 1) * 128],
                                        pt_sb[:, 0, jj0 * 128:(jj0 + 1) * 128],
                                        triT[:], mybir.AluOpType.mult))
                                    ve(nc.vector.tensor_tensor(
                                        pt_sb[:, 1, jj1 * 128:(jj1 + 1) * 128],
                                        pt_sb[:, 1, jj1 * 128:(jj1 + 1) * 128],
                                        triT[:], mybir.AluOpType.mult))
                                # pair-sum for the softmax denominator
                                padd = att.tile([128, 512], BF16,
                                                name="padd", tag="padd",
                                                bufs=6)
                                if jj0 < 0:
                                    ve(nc.vector.tensor_tensor(
                                        padd[:], pt_sb[:, 0, :],
                                        pt_sb[:, 1, :], mybir.AluOpType.add))
                                else:
                                    ve(nc.vector.tensor_copy(
                                        padd[:, c00:c01],
                                        pt_sb[:, 0, c00:c01]))
                                    ve(nc.vector.tensor_tensor(
                                        padd[:, c01:512],
                                        pt_sb[:, 0, c01:512],
                                        pt_sb[:, 1, c01:512],
                                        mybir.AluOpType.add))
                                if pr == nkt // 2 - 1:
                                    # unit boundary for the PE chain
                                    unit_last[0] = sc2
                                    unit_first[0] = None
                                inflight.append(
                                    {"u": u, "pr": pr, "kt0": kt0,
                                     "kt1": kt1, "c00": c00, "c01": c01,
                                     "pt": pt_sb, "padd": padd})
                                while len(inflight) > LAG:
                                    emit_consumers(inflight.pop(0))
                while inflight:
                    emit_consumers(inflight.pop(0))

                # ====== o_proj (contraction split by head-half) ==============
                with tc.tile_pool(name="proj", bufs=1) as proj:
                    at1_sb = proj.tile([128, NCORES, RPC], BF16)
                    at2_sb = proj.tile([128, NCORES, RPC], BF16)
                    for r in range(NCORES):
                        nc.sync.dma_start(at1_sb[:, r, :], a2a_out1[r, :, :])
                    s1_sb = proj.tile([128, 16, 512], BF16)

                    def part1_unit(ti):
                        mp, nn = ti // 4, ti % 4
                        ps_a = ps2.tile([128, 512], F32, name="ps_a",
                                         tag="ops", bufs=2)
                        for r in range(NCORES):
                            pe(nc.tensor.matmul(
                                ps_a[:],
                                at1_sb[:, r, mp * 128:(mp + 1) * 128],
                                woe_sb[:, r, nn * 512:(nn + 1) * 512],
                                start=(r == 0), stop=(r == NCORES - 1)))
                        close_unit()
                        ve(nc.vector.tensor_copy(s1_sb[:, ti, :], ps_a[:]))
                        close_vunit()

                    for ti in range(16):
                        part1_unit(ti)

                    # part 2 (h1 contraction) + output
                    for r in range(NCORES):
                        nc.scalar.dma_start(at2_sb[:, r, :],
                                            a2a_out2[r, :, :])
                    for nn in range(H // 512):
                        for mp in range(RPC // 128):
                            ti = mp * 4 + nn
                            ps_b = ps2.tile([128, 512], F32, name="ps_b",
                                             tag="ops", bufs=2)
                            for r in range(NCORES):
                                pe(nc.tensor.matmul(
                                    ps_b[:],
                                    at2_sb[:, r, mp * 128:(mp + 1) * 128],
                                    woo_sb[:, r, nn * 512:(nn + 1) * 512],
                                    start=(r == 0), stop=(r == NCORES - 1)))
                            close_unit()
                            ev = proj.tile([128, 512], BF16, name="ev",
                                           tag="ev", bufs=4)
                            ve(nc.vector.scalar_tensor_tensor(
                                ev[:], ps_b[:], 1.0, s1_sb[:, ti, :],
                                mybir.AluOpType.mult, mybir.AluOpType.add))
                            close_vunit()
                            oeng = nc.sync if ti % 2 == 0 else nc.scalar
                            oeng.dma_start(
                                out.ap()[mp * 128:(mp + 1) * 128,
                                         nn * 512:(nn + 1) * 512], ev[:])
                astack.close()

    nc.compile()
    return nc


def _get_nc():
    if "nc" not in _CACHE:
        _CACHE["nc"] = _build()
    return _CACHE["nc"]


def _prep_inputs(x, cos, sin, wq, wk, wv, wo):
    bf = ml_dtypes.bfloat16
    xf = np.asarray(x, np.float32).reshape(POS, H)
    # [PT_N, 128, KT, 512]: xTt[pt,p,k,j] = x[pt*512+j, k*128+p]
    # (partition-major: each tile loads as one contiguous 2D DMA)
    xT = np.ascontiguousarray(
        xf.reshape(PT_N, 512, KT, 128).transpose(0, 3, 2, 1)).astype(bf)
    cosT = np.ascontiguousarray(np.asarray(cos, np.float32).T).astype(bf)
    sinT = np.asarray(sin, np.float32).T.copy()
    sinT[0:64, :] = -sinT[0:64, :]
    sinT = np.ascontiguousarray(sinT).astype(bf)
    # wo split even/odd contraction tiles, partition-major:
    # wo_b[e, p, kk, m] = wo[(2*kk+e)*128 + p, m]
    wo_r = np.asarray(wo, np.float32).reshape(KT, 128, H)
    wo_b = np.ascontiguousarray(
        np.stack([wo_r[0::2], wo_r[1::2]], axis=0).transpose(0, 2, 1, 3)
    ).astype(bf)
    wq = np.asarray(wq, np.float32)
    wk = np.asarray(wk, np.float32)
    wv = np.asarray(wv, np.float32)
    sel0 = np.zeros((128, 512), np.uint8)
    sel1 = np.ones((128, 512), np.uint8)

    in_maps = []
    for i in range(NCORES):
        kv = i // 2
        half = i % 2
        xp = np.ascontiguousarray(np.concatenate(
            [xT[half * HPT:(half + 1) * HPT],
             xT[(1 - half) * HPT:(2 - half) * HPT]], axis=0))
        in_maps.append({
            "xT": xp,
            "wq": np.ascontiguousarray(
                wq[:, i * HPC * HD:(i + 1) * HPC * HD].reshape(
                    KT, 128, HPC * HD).transpose(1, 0, 2)).astype(bf),
            "wk": np.ascontiguousarray(
                wk[:, kv * HD:(kv + 1) * HD].reshape(
                    KT, 128, HD).transpose(1, 0, 2)).astype(bf),
            "wv": np.ascontiguousarray(
                wv[:, kv * HD:(kv + 1) * HD].reshape(
                    KT, 128, HD).transpose(1, 0, 2)).astype(bf),
            "cosT": cosT,
            "ssinT": sinT,
            "qsel": sel1 if half else sel0,
            "wo": wo_b,
        })
    return in_maps


def kernel(x, cos, sin, wq, wk, wv, wo, _trace=False):
    nc = _get_nc()
    in_maps = _prep_inputs(x, cos, sin, wq, wk, wv, wo)
    res = run_bass_kernel_spmd(nc, in_maps, core_ids=list(range(NCORES)),
                               trace=_trace)
    rows = np.concatenate([np.asarray(res.results[i]["out"], np.float32)
                           for i in range(NCORES)], axis=0)
    out = rows.reshape(B, S, H)
    if _trace:
        _CACHE["last_exec_time_ns"] = res.exec_time_ns
        _CACHE["last_results"] = res
    return out

